# revision 15
# baseline (speedup 1.0000x reference)
"""Mamba block Trainium2 kernel, 8-way tensor-parallel over d_inner.

Shapes (hardcoded from the problem spec):
  hidden_states [2, 1024, 1024], d_model=1024, d_inner=2048, d_state=16,
  dt_rank=64, d_conv=4.  Each core owns DL=256 d_inner channels.

Per-core dataflow:
  in_proj (fp16 matmuls, fp32 accumulate) -> x (padded conv layout) / silu(z)
  causal depthwise conv (DVE/POOL shifted MACs) + silu -> u
  x_proj partial (fp32) -> AllReduce#1 [96, 2048] -> xdbl dt rows on SBUF
  dt_proj (fp32); softplus via -ln(sigmoid(-x)) -> delta tile holds ln(sigmoid)
  per (b, n): dA = exp(+exp(A_log)[:,n] * lnsig) (ACT per-partition scale)
              B_bc/C_bc row broadcast [1,l]->[128,l] via DMA from AR1 DRAM
              dbu = du * B_bc (DVE);  h = scan(dA, dbu) (native DVE scan)
              ch = h * C_bc (DVE/POOL);  psum_y += I @ ch (PE bf16 accumulate)
  y = (u*D + y) * silu(z) -> fp16;  out_proj partial (fp16) -> AllReduce#2
"""
import sys
sys.path.insert(0, "/opt/trn_rl_repo")
import numpy as np

import concourse.bass as bass
import concourse.bacc as bacc
import concourse.mybir as mybir
import concourse.tile as tile
from concourse.tile import ScopedClock, VectorClock

F32 = mybir.dt.float32
F16 = mybir.dt.float16
BF16 = mybir.dt.bfloat16
AF = mybir.ActivationFunctionType
OP = mybir.AluOpType

B, L, D, DI, NST, RNK, KC = 2, 1024, 1024, 2048, 16, 64, 4
NC_ = 8
DL = DI // NC_          # 256 local channels
T = B * L               # 2048 tokens


def _patch_drain(tc_mod):
    """This walrus build allows only one sem wait per CTRL Drain; split the
    TileContext exit drain into one drain per proc."""
    def _drain_and_barrier(self, tick_clock, wait_clock):
        gc = tick_clock.global_clock
        ticks = list(gc)
        for proc, t in enumerate(ticks):
            if t > 0:
                vec = [0] * len(ticks)
                vec[proc] = t
                sub = ScopedClock({None: VectorClock(vec)})
                d = self.nc.sync.drain()
                wait_clock.add_sem_waits(d.ins, sub)
        self.nc.all_engine_barrier()
        assert self.sems is not None
        popped = self.nc._tile_sem_poison_stack.pop()
        assert popped is self._sem_poison
        self.nc.clear_and_free_semaphores(list(self.sems.allocated().values()))
        self.nc.all_engine_barrier()
    tc_mod.TileContext._drain_and_barrier = _drain_and_barrier




def build_nc():
    nc = bacc.Bacc()
    dp = nc.declare_dram_parameter
    hsT = dp("hsT", [D, T], F16, isOutput=False)             # hidden^T fp16
    wxz = dp("wxzT", [8, 128, 512], F16, isOutput=False)     # in_proj^T k-tiles
    xpw = dp("xpwT", [2, 128, 96], F32, isOutput=False)      # x_proj^T k-tiles
    dtw = dp("dtwT", [RNK, DL], F32, isOutput=False)         # dt_proj^T
    wo = dp("woT", [2, 128, D], F16, isOutput=False)         # out_proj^T k-tiles
    cw = dp("convw", [2, 128, KC], F32, isOutput=False)
    cb = dp("convb", [2, 128, 1], F32, isOutput=False)
    db2 = dp("dtb2n", [2, 128, 1], F32, isOutput=False)      # -2*dt_proj_b
    av = dp("Apos", [2, 128, NST], F32, isOutput=False)      # +exp(A_log)
    dv = dp("Dvec", [2, 128, 1], F32, isOutput=False)
    idm = dp("ident", [128, 128], F32, isOutput=False)
    out = dp("out", [T, D], F32, isOutput=True)

    ar1_in = nc.dram_tensor("ar1_in", [96, T], F32)
    ar1_out = nc.dram_tensor("ar1_out", [96, T], F32, addr_space="Shared")
    ar2_in = nc.dram_tensor("ar2_in", [T, D], F32)
    ar2_out = nc.dram_tensor("ar2_out", [T, D], F32, addr_space="Shared")

    PADL = L + KC - 1  # 1027 per batch segment

    with tile.TileContext(nc) as tc:
        with tc.tile_pool(name="wp", bufs=1) as wp, \
             tc.tile_pool(name="data", bufs=1) as dpool, \
             tc.tile_pool(name="stream", bufs=3) as stream, \
             tc.tile_pool(name="scan", bufs=2) as scp, \
             tc.tile_pool(name="stage", bufs=2) as stg, \
             tc.tile_pool(name="ps", bufs=4, space="PSUM") as ps, \
             tc.tile_pool(name="psy", bufs=2, space="PSUM") as psy:

            # ---- weights / constants ----
            wxz_sb = wp.tile([128, 8 * 512], F16)
            for k in range(8):
                nc.sync.dma_start(wxz_sb[:, 512 * k:512 * (k + 1)], wxz[k])
            xpw_sb = wp.tile([128, 2 * 96], F32)
            dtw_sb = wp.tile([RNK, DL], F32)
            nc.sync.dma_start(dtw_sb[:], dtw[:])
            wo_sb = wp.tile([128, 2 * D], F16)
            cw_sb = wp.tile([128, 2 * KC], F32)
            cb_sb = wp.tile([128, 2], F32)
            db2_sb = wp.tile([128, 2], F32)
            av_sb = wp.tile([128, 2 * NST], F32)
            dv_sb = wp.tile([128, 2], F32)
            for k in range(2):
                nc.sync.dma_start(xpw_sb[:, 96 * k:96 * (k + 1)], xpw[k])
                nc.sync.dma_start(wo_sb[:, D * k:D * (k + 1)], wo[k])
                nc.sync.dma_start(cw_sb[:, KC * k:KC * (k + 1)], cw[k])
                nc.sync.dma_start(cb_sb[:, k:k + 1], cb[k])
                nc.sync.dma_start(db2_sb[:, k:k + 1], db2[k])
                nc.sync.dma_start(av_sb[:, NST * k:NST * (k + 1)], av[k])
                nc.sync.dma_start(dv_sb[:, k:k + 1], dv[k])
            id_sb = wp.tile([128, 128], F32)
            nc.sync.dma_start(id_sb[:], idm[:])
            idb = wp.tile([128, 128], BF16)
            nc.vector.tensor_copy(idb[:], id_sb[:])

            # ---- persistent activations ----
            xpad = [dpool.tile([128, B * PADL], F32, name=f"xpad{i}") for i in range(2)]
            zs = [dpool.tile([128, T], F32, name=f"zs{i}") for i in range(2)]
            u = [dpool.tile([128, T], F32, name=f"u{i}") for i in range(2)]
            delta = [dpool.tile([128, T], F32, name=f"delta{i}") for i in range(2)]
            du = [dpool.tile([128, T], BF16, name=f"du{i}") for i in range(2)]
            # y reuses cy0; fp16 gated output aliases cy1's buffer (bitcast)
            cy0 = [dpool.tile([128, T], F32, name=f"cy0_{i}") for i in range(2)]
            cy1 = [dpool.tile([128, T], F32, name=f"cy1_{i}") for i in range(2)]
            y = cy0
            yg16 = [cy1[i].bitcast(F16)[:, 0:T] for i in range(2)]
            xdbl = dpool.tile([RNK, T], F32)

            cwv = cw_sb.rearrange("p (k m) -> p k m", k=2)
            wxzv = wxz_sb.rearrange("p (k m) -> p k m", k=8)
            wov = wo_sb.rearrange("p (k m) -> p k m", k=2)

            # zero the conv pads
            for dt_ in range(2):
                xp3 = xpad[dt_].rearrange("p (s l) -> p s l", s=B)
                nc.vector.memset(xp3[:, :, 0:KC - 1], 0.0)

            # ---- phase 1: in_proj (k outer, 4 live psum accumulators) ----
            for tb in range(4):
                psx = [ps.tile([128, 512], F32, name=f"psx{i}", tag="ps")
                       for i in range(4)]
                for k in range(8):
                    hkt = stream.tile([128, 512], F16, name="hkt", tag="hst")
                    nc.sync.dma_start(
                        hkt[:], hsT[128 * k:128 * (k + 1), 512 * tb:512 * (tb + 1)])
                    for half in range(4):  # x0 x1 z0 z1
                        nc.tensor.matmul(
                            psx[half][:],
                            wxzv[:, k, 128 * half:128 * (half + 1)],
                            hkt[:],
                            start=(k == 0), stop=(k == 7))
                for half in range(4):
                    dt_ = half % 2
                    if half < 2:   # x -> padded conv layout
                        b_ = tb // 2
                        off = b_ * PADL + (KC - 1) + 512 * (tb % 2)
                        nc.scalar.copy(xpad[dt_][:, off:off + 512], psx[half][:])
                    else:          # z -> silu(z) = z * sigmoid(z)
                        sg = stream.tile([128, 512], F32, name="sg", tag="sg")
                        nc.scalar.activation(sg[:], psx[half][:], AF.Sigmoid)
                        nc.vector.tensor_mul(
                            zs[dt_][:, 512 * tb:512 * (tb + 1)], psx[half][:], sg[:])

            # ---- phase 2: depthwise causal conv + silu -> u ----
            for dt_ in range(2):
                eng = nc.vector
                xp3 = xpad[dt_].rearrange("p (s l) -> p s l", s=B)
                c0_3 = cy0[dt_].rearrange("p (s l) -> p s l", s=B)
                c1_3 = cy1[dt_].rearrange("p (s l) -> p s l", s=B)
                eng.tensor_scalar_mul(c0_3[:], xp3[:, :, 0:L], cwv[:, dt_, 0:1])
                abuf = [c0_3, c1_3]
                for k in range(1, KC):
                    eng.scalar_tensor_tensor(
                        abuf[k % 2][:], xp3[:, :, k:k + L], cwv[:, dt_, k:k + 1],
                        abuf[(k + 1) % 2][:], op0=OP.mult, op1=OP.add)
                acc = abuf[(KC - 1) % 2]           # cy1
                sgt = abuf[KC % 2]                 # cy0, free after last MAC
                nc.scalar.activation(sgt.rearrange("p s l -> p (s l)")[:],
                                     acc.rearrange("p s l -> p (s l)")[:],
                                     AF.Sigmoid, bias=cb_sb[:, dt_:dt_ + 1])
                nc.vector.scalar_tensor_tensor(
                    u[dt_][:], acc.rearrange("p s l -> p (s l)")[:],
                    cb_sb[:, dt_:dt_ + 1], sgt.rearrange("p s l -> p (s l)")[:],
                    op0=OP.add, op1=OP.mult)

            # ---- phase 3: x_proj partial (fp32) -> AllReduce#1 ----
            for tb in range(4):
                ps96 = ps.tile([96, 512], F32, name="ps96", tag="ps")
                for k in range(2):
                    nc.tensor.matmul(
                        ps96[:], xpw_sb[:, 96 * k:96 * (k + 1)],
                        u[k][:, 512 * tb:512 * (tb + 1)],
                        start=(k == 0), stop=(k == 1))
                st = stg.tile([96, 512], F32, name="st_xp", tag="xp")
                nc.scalar.copy(st[:], ps96[:])
                nc.sync.dma_start(ar1_in[:, 512 * tb:512 * (tb + 1)], st[:])
            nc.gpsimd.collective_compute(
                "AllReduce", OP.add,
                replica_groups=[list(range(NC_))],
                ins=[ar1_in[:]], outs=[ar1_out[:]])
            nc.sync.dma_start(xdbl[:], ar1_out[0:RNK, :])

            # ---- phase 4: dt_proj (fp32); delta tile := ln(sigmoid(-(dt+2b)))
            for tb in range(4):
                for dt_ in range(2):
                    psd = ps.tile([128, 512], F32, name="psd", tag="ps")
                    nc.tensor.matmul(
                        psd[:], dtw_sb[:, 128 * dt_:128 * (dt_ + 1)],
                        xdbl[:, 512 * tb:512 * (tb + 1)],
                        start=True, stop=True)
                    sgd = stream.tile([128, 512], F32, name="sgd", tag="sg")
                    nc.scalar.activation(sgd[:], psd[:], AF.Sigmoid,
                                         scale=-1.0, bias=db2_sb[:, dt_:dt_ + 1])
                    nc.scalar.activation(
                        delta[dt_][:, 512 * tb:512 * (tb + 1)], sgd[:], AF.Ln)
            # du = delta*u = (-lnsig)*u   (bf16)
            for dt_ in range(2):
                nc.vector.scalar_tensor_tensor(
                    du[dt_][:], delta[dt_][:], -1.0, u[dt_][:],
                    op0=OP.mult, op1=OP.mult)

            # ---- phase 5: selective scan ----
            for b_ in range(2):
                tsl = slice(L * b_, L * (b_ + 1))
                for dt_ in range(2):
                    py0 = psy.tile([128, 512], F32, name="py0", tag="psy")
                    py1 = psy.tile([128, 512], F32, name="py1", tag="psy")
                    for n in range(NST):
                        bbc = scp.tile([128, L], F32, name="bbc", tag="bbc")
                        nc.sync.dma_start(
                            bbc[:],
                            ar1_out[RNK + n:RNK + n + 1, tsl].broadcast_to((128, L)))
                        dA = scp.tile([128, L], F32, name="dA", tag="dA")
                        nc.scalar.activation(
                            dA[:], delta[dt_][:, tsl], AF.Exp,
                            scale=av_sb[:, NST * dt_ + n:NST * dt_ + n + 1])
                        dbu = scp.tile([128, L], BF16, name="dbu", tag="dbu")
                        nc.vector.tensor_mul(dbu[:], du[dt_][:, tsl], bbc[:])
                        h = scp.tile([128, L], F32, name="h", tag="h")
                        nc.vector.tensor_tensor_scan(
                            h[:], dA[:], dbu[:], 0.0, op0=OP.mult, op1=OP.add)
                        cbc = scp.tile([128, L], F32, name="cbc", tag="cbc")
                        nc.sync.dma_start(
                            cbc[:],
                            ar1_out[RNK + NST + n:RNK + NST + n + 1, tsl]
                            .broadcast_to((128, L)))
                        ch = scp.tile([128, L], BF16, name="ch", tag="ch")
                        cheng = nc.vector if dt_ == 0 else nc.gpsimd
                        cheng.tensor_mul(ch[:], h[:], cbc[:])
                        nc.tensor.matmul(py0[:], idb[:], ch[:, 0:512],
                                         start=(n == 0), stop=(n == NST - 1))
                        nc.tensor.matmul(py1[:], idb[:], ch[:, 512:L],
                                         start=(n == 0), stop=(n == NST - 1))
                    nc.scalar.copy(y[dt_][:, L * b_:L * b_ + 512], py0[:])
                    nc.scalar.copy(y[dt_][:, L * b_ + 512:L * (b_ + 1)], py1[:])

            # ---- phase 6: skip + gate (gate output fp16, aliases cy1) ----
            for dt_ in range(2):
                nc.vector.scalar_tensor_tensor(
                    y[dt_][:], u[dt_][:], dv_sb[:, dt_:dt_ + 1], y[dt_][:],
                    op0=OP.mult, op1=OP.add)
                nc.vector.tensor_mul(yg16[dt_][:], y[dt_][:], zs[dt_][:])

            # ---- phase 7: out_proj partial (fp16) -> AllReduce#2 -> out ----
            for tt in range(16):
                for mb in range(2):
                    pso = ps.tile([128, 512], F32, name="pso", tag="ps")
                    for k in range(2):
                        nc.tensor.matmul(
                            pso[:],
                            yg16[k][:, 128 * tt:128 * (tt + 1)],
                            wov[:, k, 512 * mb:512 * (mb + 1)],
                            start=(k == 0), stop=(k == 1))
                    st = stg.tile([128, 512], F32, name="st_op", tag="op")
                    nc.scalar.copy(st[:], pso[:])
                    nc.sync.dma_start(
                        ar2_in[128 * tt:128 * (tt + 1), 512 * mb:512 * (mb + 1)],
                        st[:])
            nc.gpsimd.collective_compute(
                "AllReduce", OP.add,
                replica_groups=[list(range(NC_))],
                ins=[ar2_in[:]], outs=[ar2_out[:]])
            nc.sync.dma_start(out[:], ar2_out[:])
    nc.finalize()
    return nc


def make_in_maps(inputs):
    hs = np.asarray(inputs["hidden_states"], np.float32)
    ipw = np.asarray(inputs["in_proj_w"], np.float32)
    cw = np.asarray(inputs["conv_w"], np.float32)
    cb = np.asarray(inputs["conv_b"], np.float32)
    xpw = np.asarray(inputs["x_proj_w"], np.float32)
    dtw = np.asarray(inputs["dt_proj_w"], np.float32)
    dtb = np.asarray(inputs["dt_proj_b"], np.float32)
    alog = np.asarray(inputs["A_log"], np.float32)
    dvec = np.asarray(inputs["D"], np.float32)
    wo = np.asarray(inputs["out_proj_w"], np.float32)

    hsT = np.ascontiguousarray(hs.transpose(2, 0, 1).reshape(D, T)).astype(np.float16)
    ident = np.eye(128, dtype=np.float32)

    in_maps = []
    for c in range(NC_):
        sl = slice(DL * c, DL * (c + 1))
        wxzT = np.concatenate([ipw[sl].T, ipw[DI + DL * c: DI + DL * (c + 1)].T],
                              axis=1)                      # [1024, 512]
        m = {
            "hsT": hsT,
            "wxzT": np.ascontiguousarray(wxzT.reshape(8, 128, 512)).astype(np.float16),
            "xpwT": np.ascontiguousarray(xpw[:, sl].T.reshape(2, 128, 96)),
            "dtwT": np.ascontiguousarray(dtw[sl].T),        # [64, 256]
            "woT": np.ascontiguousarray(wo[:, sl].T.reshape(2, 128, D)).astype(np.float16),
            "convw": np.ascontiguousarray(cw[sl, 0, :].reshape(2, 128, KC)),
            "convb": np.ascontiguousarray(cb[sl].reshape(2, 128, 1)),
            "dtb2n": np.ascontiguousarray((-2.0 * dtb[sl]).reshape(2, 128, 1)),
            "Apos": np.ascontiguousarray(np.exp(alog[sl]).reshape(2, 128, NST)),
            "Dvec": np.ascontiguousarray(dvec[sl].reshape(2, 128, 1)),
            "ident": ident,
        }
        in_maps.append(m)
    return in_maps


def kernel(**inputs):
    import os
    from concourse.bass_utils import run_bass_kernel_spmd
    nc = build_nc()
    in_maps = make_in_maps(inputs)
    trace = bool(int(os.environ.get("MAMBA_TRACE", "0")))
    res = run_bass_kernel_spmd(nc, in_maps, list(range(NC_)), trace=trace)
    if trace and res.exec_time_ns is not None:
        print(f"HW exec time: {res.exec_time_ns} ns")
    out = np.asarray(res.results[0]["out"], np.float32).reshape(B, L, D)
    return out


# revision 16
# speedup vs baseline: 1.9953x; 1.9953x over previous
"""Mamba block Trainium2 kernel, 8-way tensor-parallel over d_inner.

Shapes (hardcoded from the problem spec):
  hidden_states [2, 1024, 1024], d_model=1024, d_inner=2048, d_state=16,
  dt_rank=64, d_conv=4.  Each core owns DL=256 d_inner channels.

Per-core dataflow:
  in_proj (fp16 matmuls, fp32 accumulate) -> x (padded conv layout) / silu(z)
  causal depthwise conv (DVE/POOL shifted MACs) + silu -> u
  x_proj partial (fp32) -> AllReduce#1 [96, 2048] -> xdbl dt rows on SBUF
  dt_proj (fp32); softplus via -ln(sigmoid(-x)) -> delta tile holds ln(sigmoid)
  per (b, n): dA = exp(+exp(A_log)[:,n] * lnsig) (ACT per-partition scale)
              B_bc/C_bc row broadcast [1,l]->[128,l] via DMA from AR1 DRAM
              dbu = du * B_bc (DVE);  h = scan(dA, dbu) (native DVE scan)
              ch = h * C_bc (DVE/POOL);  psum_y += I @ ch (PE bf16 accumulate)
  y = (u*D + y) * silu(z) -> fp16;  out_proj partial (fp16) -> AllReduce#2
"""
import sys, os
sys.path.insert(0, "/opt/trn_rl_repo")
import numpy as np

import concourse.bass as bass
import concourse.bacc as bacc
import concourse.mybir as mybir
import concourse.tile as tile
from concourse.tile import ScopedClock, VectorClock

F32 = mybir.dt.float32
F16 = mybir.dt.float16
BF16 = mybir.dt.bfloat16
AF = mybir.ActivationFunctionType
OP = mybir.AluOpType

B, L, D, DI, NST, RNK, KC = 2, 1024, 1024, 2048, 16, 64, 4
NC_ = 8
DL = DI // NC_          # 256 local channels
T = B * L               # 2048 tokens


def _patch_drain(tc_mod):
    """This walrus build allows only one sem wait per CTRL Drain; split the
    TileContext exit drain into one drain per proc."""
    def _drain_and_barrier(self, tick_clock, wait_clock):
        gc = tick_clock.global_clock
        ticks = list(gc)
        for proc, t in enumerate(ticks):
            if t > 0:
                vec = [0] * len(ticks)
                vec[proc] = t
                sub = ScopedClock({None: VectorClock(vec)})
                d = self.nc.sync.drain()
                wait_clock.add_sem_waits(d.ins, sub)
        self.nc.all_engine_barrier()
        assert self.sems is not None
        popped = self.nc._tile_sem_poison_stack.pop()
        assert popped is self._sem_poison
        self.nc.clear_and_free_semaphores(list(self.sems.allocated().values()))
        self.nc.all_engine_barrier()
    tc_mod.TileContext._drain_and_barrier = _drain_and_barrier




def build_nc():
    nc = bacc.Bacc()
    dp = nc.declare_dram_parameter
    hsT = dp("hsT", [D, T], F16, isOutput=False)             # hidden^T fp16
    wxz = dp("wxzT", [8, 128, 512], F16, isOutput=False)     # in_proj^T k-tiles
    xpw = dp("xpwT", [2, 128, 96], F32, isOutput=False)      # x_proj^T k-tiles
    dtw = dp("dtwT", [RNK, DL], F32, isOutput=False)         # dt_proj^T
    wo = dp("woT", [2, 128, D], F16, isOutput=False)         # out_proj^T k-tiles
    cw = dp("convw", [2, 128, KC], F32, isOutput=False)
    cb = dp("convb", [2, 128, 1], F32, isOutput=False)
    db2 = dp("dtb2n", [2, 128, 1], F32, isOutput=False)      # -2*dt_proj_b
    av = dp("Apos", [2, 128, NST], F32, isOutput=False)      # +exp(A_log)
    dv = dp("Dvec", [2, 128, 1], F32, isOutput=False)
    idm = dp("ident", [128, 128], F32, isOutput=False)
    out = dp("out", [T, D], F32, isOutput=True)

    ar1_in = nc.dram_tensor("ar1_in", [96, T], F32)
    ar1_out = nc.dram_tensor("ar1_out", [96, T], F32, addr_space="Shared")
    ar2_in = nc.dram_tensor("ar2_in", [T, D], F32)
    ar2_out = nc.dram_tensor("ar2_out", [T, D], F32, addr_space="Shared")

    PADL = L + KC - 1  # 1027 per batch segment

    with tile.TileContext(nc) as tc:
        with tc.tile_pool(name="wp", bufs=1) as wp, \
             tc.tile_pool(name="data", bufs=1) as dpool, \
             tc.tile_pool(name="stream", bufs=3) as stream, \
             tc.tile_pool(name="scan", bufs=2) as scp, \
             tc.tile_pool(name="stage", bufs=2) as stg, \
             tc.tile_pool(name="ps", bufs=4, space="PSUM") as ps, \
             tc.tile_pool(name="psy", bufs=2, space="PSUM") as psy:

            # ---- weights / constants ----
            wxz_sb = wp.tile([128, 8 * 512], F16)
            for k in range(8):
                nc.sync.dma_start(wxz_sb[:, 512 * k:512 * (k + 1)], wxz[k])
            xpw_sb = wp.tile([128, 2 * 96], F32)
            dtw_sb = wp.tile([RNK, DL], F32)
            nc.sync.dma_start(dtw_sb[:], dtw[:])
            wo_sb = wp.tile([128, 2 * D], F16)
            cw_sb = wp.tile([128, 2 * KC], F32)
            cb_sb = wp.tile([128, 2], F32)
            db2_sb = wp.tile([128, 2], F32)
            av_sb = wp.tile([128, 2 * NST], F32)
            dv_sb = wp.tile([128, 2], F32)
            for k in range(2):
                nc.sync.dma_start(xpw_sb[:, 96 * k:96 * (k + 1)], xpw[k])
                nc.sync.dma_start(wo_sb[:, D * k:D * (k + 1)], wo[k])
                nc.sync.dma_start(cw_sb[:, KC * k:KC * (k + 1)], cw[k])
                nc.sync.dma_start(cb_sb[:, k:k + 1], cb[k])
                nc.sync.dma_start(db2_sb[:, k:k + 1], db2[k])
                nc.sync.dma_start(av_sb[:, NST * k:NST * (k + 1)], av[k])
                nc.sync.dma_start(dv_sb[:, k:k + 1], dv[k])
            id_sb = wp.tile([128, 128], F32)
            nc.sync.dma_start(id_sb[:], idm[:])
            idb = wp.tile([128, 128], BF16)
            nc.vector.tensor_copy(idb[:], id_sb[:])

            # ---- persistent activations ----
            xpad = [dpool.tile([128, B * PADL], F32, name=f"xpad{i}") for i in range(2)]
            zs = [dpool.tile([128, T], F32, name=f"zs{i}") for i in range(2)]
            u = [dpool.tile([128, T], F32, name=f"u{i}") for i in range(2)]
            delta = [dpool.tile([128, T], F32, name=f"delta{i}") for i in range(2)]
            du = [dpool.tile([128, T], BF16, name=f"du{i}") for i in range(2)]
            # y reuses cy0; fp16 gated output aliases cy1's buffer (bitcast)
            cy0 = [dpool.tile([128, T], F32, name=f"cy0_{i}") for i in range(2)]
            cy1 = [dpool.tile([128, T], F32, name=f"cy1_{i}") for i in range(2)]
            y = cy0
            yg16 = [cy1[i].bitcast(F16)[:, 0:T] for i in range(2)]
            xdbl = dpool.tile([RNK, T], F32)

            cwv = cw_sb.rearrange("p (k m) -> p k m", k=2)
            wxzv = wxz_sb.rearrange("p (k m) -> p k m", k=8)
            wov = wo_sb.rearrange("p (k m) -> p k m", k=2)

            # zero the conv pads
            for dt_ in range(2):
                xp3 = xpad[dt_].rearrange("p (s l) -> p s l", s=B)
                nc.vector.memset(xp3[:, :, 0:KC - 1], 0.0)

            # ---- phase 1: in_proj (k outer, 4 live psum accumulators) ----
            for tb in range(4):
                psx = [ps.tile([128, 512], F32, name=f"psx{i}", tag="ps")
                       for i in range(4)]
                for k in range(8):
                    hkt = stream.tile([128, 512], F16, name="hkt", tag="hst")
                    nc.sync.dma_start(
                        hkt[:], hsT[128 * k:128 * (k + 1), 512 * tb:512 * (tb + 1)])
                    for half in range(4):  # x0 x1 z0 z1
                        nc.tensor.matmul(
                            psx[half][:],
                            wxzv[:, k, 128 * half:128 * (half + 1)],
                            hkt[:],
                            start=(k == 0), stop=(k == 7))
                for half in range(4):
                    dt_ = half % 2
                    if half < 2:   # x -> padded conv layout
                        b_ = tb // 2
                        off = b_ * PADL + (KC - 1) + 512 * (tb % 2)
                        nc.scalar.copy(xpad[dt_][:, off:off + 512], psx[half][:])
                    else:          # z -> silu(z) = z * sigmoid(z)
                        sg = stream.tile([128, 512], F32, name="sg", tag="sg")
                        nc.scalar.activation(sg[:], psx[half][:], AF.Sigmoid)
                        nc.vector.tensor_mul(
                            zs[dt_][:, 512 * tb:512 * (tb + 1)], psx[half][:], sg[:])

            # ---- phase 2: depthwise causal conv + silu -> u ----
            for dt_ in range(2):
                eng = nc.vector
                xp3 = xpad[dt_].rearrange("p (s l) -> p s l", s=B)
                c0_3 = cy0[dt_].rearrange("p (s l) -> p s l", s=B)
                c1_3 = cy1[dt_].rearrange("p (s l) -> p s l", s=B)
                eng.tensor_scalar_mul(c0_3[:], xp3[:, :, 0:L], cwv[:, dt_, 0:1])
                abuf = [c0_3, c1_3]
                for k in range(1, KC):
                    eng.scalar_tensor_tensor(
                        abuf[k % 2][:], xp3[:, :, k:k + L], cwv[:, dt_, k:k + 1],
                        abuf[(k + 1) % 2][:], op0=OP.mult, op1=OP.add)
                acc = abuf[(KC - 1) % 2]           # cy1
                sgt = abuf[KC % 2]                 # cy0, free after last MAC
                nc.scalar.activation(sgt.rearrange("p s l -> p (s l)")[:],
                                     acc.rearrange("p s l -> p (s l)")[:],
                                     AF.Sigmoid, bias=cb_sb[:, dt_:dt_ + 1])
                nc.vector.scalar_tensor_tensor(
                    u[dt_][:], acc.rearrange("p s l -> p (s l)")[:],
                    cb_sb[:, dt_:dt_ + 1], sgt.rearrange("p s l -> p (s l)")[:],
                    op0=OP.add, op1=OP.mult)

            # ---- phase 3: x_proj partial (fp32) -> AllReduce#1 ----
            for tb in range(4):
                ps96 = ps.tile([96, 512], F32, name="ps96", tag="ps")
                for k in range(2):
                    nc.tensor.matmul(
                        ps96[:], xpw_sb[:, 96 * k:96 * (k + 1)],
                        u[k][:, 512 * tb:512 * (tb + 1)],
                        start=(k == 0), stop=(k == 1))
                st = stg.tile([96, 512], F32, name="st_xp", tag="xp")
                nc.scalar.copy(st[:], ps96[:])
                nc.sync.dma_start(ar1_in[:, 512 * tb:512 * (tb + 1)], st[:])
            if os.environ.get("MAMBA_NO_AR"):
                nc.sync.dma_start(ar1_out[:], ar1_in[:])
            else:
                nc.gpsimd.collective_compute(
                    "AllReduce", OP.add,
                    replica_groups=[list(range(NC_))],
                    ins=[ar1_in[:]], outs=[ar1_out[:]])
            nc.sync.dma_start(xdbl[:], ar1_out[0:RNK, :])

            # ---- phase 4: dt_proj (fp32); delta tile := ln(sigmoid(-(dt+2b)))
            for tb in range(4):
                for dt_ in range(2):
                    psd = ps.tile([128, 512], F32, name="psd", tag="ps")
                    nc.tensor.matmul(
                        psd[:], dtw_sb[:, 128 * dt_:128 * (dt_ + 1)],
                        xdbl[:, 512 * tb:512 * (tb + 1)],
                        start=True, stop=True)
                    sgd = stream.tile([128, 512], F32, name="sgd", tag="sg")
                    nc.scalar.activation(sgd[:], psd[:], AF.Sigmoid,
                                         scale=-1.0, bias=db2_sb[:, dt_:dt_ + 1])
                    nc.scalar.activation(
                        delta[dt_][:, 512 * tb:512 * (tb + 1)], sgd[:], AF.Ln)
            # du = delta*u = (-lnsig)*u   (bf16)
            for dt_ in range(2):
                nc.vector.scalar_tensor_tensor(
                    du[dt_][:], delta[dt_][:], -1.0, u[dt_][:],
                    op0=OP.mult, op1=OP.mult)

            # ---- phase 5: selective scan ----
            for b_ in range(2):
                tsl = slice(L * b_, L * (b_ + 1))
                for dt_ in range(2):
                    py0 = psy.tile([128, 512], F32, name="py0", tag="psy")
                    py1 = psy.tile([128, 512], F32, name="py1", tag="psy")
                    for n in range(NST):
                        bbc = scp.tile([128, L], F32, name="bbc", tag="bbc")
                        nc.sync.dma_start(
                            bbc[:],
                            ar1_out[RNK + n:RNK + n + 1, tsl].broadcast_to((128, L)))
                        dA = scp.tile([128, L], F32, name="dA", tag="dA")
                        nc.scalar.activation(
                            dA[:], delta[dt_][:, tsl], AF.Exp,
                            scale=av_sb[:, NST * dt_ + n:NST * dt_ + n + 1])
                        dbu = scp.tile([128, L], BF16, name="dbu", tag="dbu")
                        nc.vector.tensor_mul(dbu[:], du[dt_][:, tsl], bbc[:])
                        h = scp.tile([128, L], F32, name="h", tag="h")
                        nc.vector.tensor_tensor_scan(
                            h[:], dA[:], dbu[:], 0.0, op0=OP.mult, op1=OP.add)
                        cbc = scp.tile([128, L], F32, name="cbc", tag="cbc")
                        nc.sync.dma_start(
                            cbc[:],
                            ar1_out[RNK + NST + n:RNK + NST + n + 1, tsl]
                            .broadcast_to((128, L)))
                        ch = scp.tile([128, L], BF16, name="ch", tag="ch")
                        cheng = nc.vector if dt_ == 0 else nc.gpsimd
                        cheng.tensor_mul(ch[:], h[:], cbc[:])
                        nc.tensor.matmul(py0[:], idb[:], ch[:, 0:512],
                                         start=(n == 0), stop=(n == NST - 1))
                        nc.tensor.matmul(py1[:], idb[:], ch[:, 512:L],
                                         start=(n == 0), stop=(n == NST - 1))
                    nc.scalar.copy(y[dt_][:, L * b_:L * b_ + 512], py0[:])
                    nc.scalar.copy(y[dt_][:, L * b_ + 512:L * (b_ + 1)], py1[:])

            # ---- phase 6: skip + gate (gate output fp16, aliases cy1) ----
            for dt_ in range(2):
                nc.vector.scalar_tensor_tensor(
                    y[dt_][:], u[dt_][:], dv_sb[:, dt_:dt_ + 1], y[dt_][:],
                    op0=OP.mult, op1=OP.add)
                nc.vector.tensor_mul(yg16[dt_][:], y[dt_][:], zs[dt_][:])

            # ---- phase 7: out_proj partial (fp16) -> AllReduce#2 -> out ----
            for tt in range(16):
                for mb in range(2):
                    pso = ps.tile([128, 512], F32, name="pso", tag="ps")
                    for k in range(2):
                        nc.tensor.matmul(
                            pso[:],
                            yg16[k][:, 128 * tt:128 * (tt + 1)],
                            wov[:, k, 512 * mb:512 * (mb + 1)],
                            start=(k == 0), stop=(k == 1))
                    st = stg.tile([128, 512], F32, name="st_op", tag="op")
                    nc.scalar.copy(st[:], pso[:])
                    nc.sync.dma_start(
                        ar2_in[128 * tt:128 * (tt + 1), 512 * mb:512 * (mb + 1)],
                        st[:])
            if os.environ.get("MAMBA_NO_AR"):
                nc.sync.dma_start(ar2_out[:], ar2_in[:])
            else:
                nc.gpsimd.collective_compute(
                    "AllReduce", OP.add,
                    replica_groups=[list(range(NC_))],
                    ins=[ar2_in[:]], outs=[ar2_out[:]])
            nc.sync.dma_start(out[:], ar2_out[:])
    nc.finalize()
    return nc


def make_in_maps(inputs):
    hs = np.asarray(inputs["hidden_states"], np.float32)
    ipw = np.asarray(inputs["in_proj_w"], np.float32)
    cw = np.asarray(inputs["conv_w"], np.float32)
    cb = np.asarray(inputs["conv_b"], np.float32)
    xpw = np.asarray(inputs["x_proj_w"], np.float32)
    dtw = np.asarray(inputs["dt_proj_w"], np.float32)
    dtb = np.asarray(inputs["dt_proj_b"], np.float32)
    alog = np.asarray(inputs["A_log"], np.float32)
    dvec = np.asarray(inputs["D"], np.float32)
    wo = np.asarray(inputs["out_proj_w"], np.float32)

    hsT = np.ascontiguousarray(hs.transpose(2, 0, 1).reshape(D, T)).astype(np.float16)
    ident = np.eye(128, dtype=np.float32)

    in_maps = []
    for c in range(NC_):
        sl = slice(DL * c, DL * (c + 1))
        wxzT = np.concatenate([ipw[sl].T, ipw[DI + DL * c: DI + DL * (c + 1)].T],
                              axis=1)                      # [1024, 512]
        m = {
            "hsT": hsT,
            "wxzT": np.ascontiguousarray(wxzT.reshape(8, 128, 512)).astype(np.float16),
            "xpwT": np.ascontiguousarray(xpw[:, sl].T.reshape(2, 128, 96)),
            "dtwT": np.ascontiguousarray(dtw[sl].T),        # [64, 256]
            "woT": np.ascontiguousarray(wo[:, sl].T.reshape(2, 128, D)).astype(np.float16),
            "convw": np.ascontiguousarray(cw[sl, 0, :].reshape(2, 128, KC)),
            "convb": np.ascontiguousarray(cb[sl].reshape(2, 128, 1)),
            "dtb2n": np.ascontiguousarray((-2.0 * dtb[sl]).reshape(2, 128, 1)),
            "Apos": np.ascontiguousarray(np.exp(alog[sl]).reshape(2, 128, NST)),
            "Dvec": np.ascontiguousarray(dvec[sl].reshape(2, 128, 1)),
            "ident": ident,
        }
        in_maps.append(m)
    return in_maps


def kernel(**inputs):
    import os
    from concourse.bass_utils import run_bass_kernel_spmd
    nc = build_nc()
    in_maps = make_in_maps(inputs)
    trace = bool(int(os.environ.get("MAMBA_TRACE", "0")))
    res = run_bass_kernel_spmd(nc, in_maps, list(range(NC_)), trace=trace)
    if trace and res.exec_time_ns is not None:
        print(f"HW exec time: {res.exec_time_ns} ns")
    out = np.asarray(res.results[0]["out"], np.float32).reshape(B, L, D)
    return out


# revision 20
# speedup vs baseline: 2.3118x; 1.1587x over previous
"""Mamba block Trainium2 kernel, 8-way tensor-parallel over d_inner.

Shapes (hardcoded from the problem spec):
  hidden_states [2, 1024, 1024], d_model=1024, d_inner=2048, d_state=16,
  dt_rank=64, d_conv=4.  Each core owns DL=256 d_inner channels.

Per-core dataflow:
  in_proj (fp16 matmuls, fp32 accumulate) -> x (padded conv layout), z
  silu via tanh: silu(v) = (tanh(v/2)+1) * (v/2)    [exp_and_others set]
  causal depthwise conv (DVE shifted MACs) + silu -> u
  x_proj partial (fp32) -> AllReduce#1 [96, 2048];  B/C rows -> fp16 DRAM
  dt_proj (fp32); softplus = Ln(Exp(dt+2b) + 1)     [ln/exp sets]
  per (b, n): B_bc/C_bc fp16 row-broadcast DMA (reused by both d-tiles)
     per d-tile: dA = exp(-exp(A_log)[:,n]*delta) (ACT per-partition scale)
                 dbu = du*B_bc (DVE 2x);  h = scan(dA, dbu) (DVE native scan)
                 ch = h*C_bc (DVE/POOL);  psum_y += I @ ch (PE bf16)
  per b: y = (u*D + y) * silu(z) -> fp16; out_proj (fp16) ->
         ReduceScatter fp16 -> per-core [128, 1024] slice of each batch
"""
import sys, os
sys.path.insert(0, "/opt/trn_rl_repo")
import numpy as np

import concourse.bass as bass
import concourse.bacc as bacc
import concourse.mybir as mybir
import concourse.tile as tile

F32 = mybir.dt.float32
F16 = mybir.dt.float16
BF16 = mybir.dt.bfloat16
AF = mybir.ActivationFunctionType
OP = mybir.AluOpType

B, L, D, DI, NST, RNK, KC = 2, 1024, 1024, 2048, 16, 64, 4
NC_ = 8
DL = DI // NC_          # 256 local channels
T = B * L               # 2048 tokens


def build_nc():
    nc = bacc.Bacc()
    dp = nc.declare_dram_parameter
    hsT = dp("hsT", [D, T], F16, isOutput=False)             # hidden^T fp16
    wxz = dp("wxzT", [8, 128, 512], F16, isOutput=False)     # in_proj^T k-tiles
    xpw = dp("xpwT", [2, 128, 96], F32, isOutput=False)      # x_proj^T k-tiles
    dtw = dp("dtwT", [RNK, DL], F32, isOutput=False)         # dt_proj^T
    wo = dp("woT", [2, 128, D], F16, isOutput=False)         # out_proj^T k-tiles
    cw = dp("convw", [2, 128, KC], F32, isOutput=False)
    cbh = dp("convbh", [2, 128, 1], F32, isOutput=False)     # conv_b / 2
    db2 = dp("dtb2p", [2, 128, 1], F32, isOutput=False)      # +2*dt_proj_b
    av = dp("Aneg", [2, 128, NST], F32, isOutput=False)      # -exp(A_log)
    dv = dp("Dvec", [2, 128, 1], F32, isOutput=False)
    idm = dp("ident", [128, 128], F32, isOutput=False)
    outp = dp("out", [B, 128, D], F16, isOutput=True)        # per-core RS slice

    ar1_in = nc.dram_tensor("ar1_in", [96, T], F32)
    ar1_out = nc.dram_tensor("ar1_out", [96, T], F32, addr_space="Shared")
    bc16d = nc.dram_tensor("bc16d", [32, T], F16)            # B/C rows fp16
    ar2_in = [nc.dram_tensor(f"ar2_in{b}", [L, D], F16) for b in range(B)]
    ar2_out = [nc.dram_tensor(f"ar2_out{b}", [128, D], F16) for b in range(B)]

    PADL = L + KC - 1  # 1027 per batch segment

    with tile.TileContext(nc) as tc:
        with tc.tile_pool(name="wp", bufs=1) as wp, \
             tc.tile_pool(name="data", bufs=1) as dpool, \
             tc.tile_pool(name="stream", bufs=3) as stream, \
             tc.tile_pool(name="scan", bufs=2) as scp, \
             tc.tile_pool(name="stage", bufs=2) as stg, \
             tc.tile_pool(name="ps", bufs=4, space="PSUM") as ps, \
             tc.tile_pool(name="psy", bufs=4, space="PSUM") as psy:

            # ---- weights / constants ----
            wxz_sb = wp.tile([128, 8 * 512], F16)
            for k in range(8):
                nc.sync.dma_start(wxz_sb[:, 512 * k:512 * (k + 1)], wxz[k])
            xpw_sb = wp.tile([128, 2 * 96], F32)
            dtw_sb = wp.tile([RNK, DL], F32)
            nc.sync.dma_start(dtw_sb[:], dtw[:])
            wo_sb = wp.tile([128, 2 * D], F16)
            cw_sb = wp.tile([128, 2 * KC], F32)
            cbh_sb = wp.tile([128, 2], F32)
            db2_sb = wp.tile([128, 2], F32)
            av_sb = wp.tile([128, 2 * NST], F32)
            dv_sb = wp.tile([128, 2], F32)
            for k in range(2):
                nc.sync.dma_start(xpw_sb[:, 96 * k:96 * (k + 1)], xpw[k])
                nc.sync.dma_start(wo_sb[:, D * k:D * (k + 1)], wo[k])
                nc.sync.dma_start(cw_sb[:, KC * k:KC * (k + 1)], cw[k])
                nc.sync.dma_start(cbh_sb[:, k:k + 1], cbh[k])
                nc.sync.dma_start(db2_sb[:, k:k + 1], db2[k])
                nc.sync.dma_start(av_sb[:, NST * k:NST * (k + 1)], av[k])
                nc.sync.dma_start(dv_sb[:, k:k + 1], dv[k])
            id_sb = wp.tile([128, 128], F32)
            nc.sync.dma_start(id_sb[:], idm[:])
            idb = wp.tile([128, 128], BF16)
            nc.vector.tensor_copy(idb[:], id_sb[:])

            # ---- persistent activations ----
            xpad = [dpool.tile([128, B * PADL], F32, name=f"xpad{i}") for i in range(2)]
            zs = [dpool.tile([128, T], F32, name=f"zs{i}") for i in range(2)]
            u = [dpool.tile([128, T], F32, name=f"u{i}") for i in range(2)]
            delta = [dpool.tile([128, T], F32, name=f"delta{i}") for i in range(2)]
            du = [dpool.tile([128, T], F16, name=f"du{i}") for i in range(2)]
            # y reuses cy0; fp16 gated output aliases cy1's buffer (bitcast)
            cy0 = [dpool.tile([128, T], F32, name=f"cy0_{i}") for i in range(2)]
            cy1 = [dpool.tile([128, T], F32, name=f"cy1_{i}") for i in range(2)]
            y = cy0
            yg16 = [cy1[i].bitcast(F16)[:, 0:T] for i in range(2)]
            xdbl = dpool.tile([RNK, T], F32)
            bc16_sb = dpool.tile([32, T], F16)

            cwv = cw_sb.rearrange("p (k m) -> p k m", k=2)
            wxzv = wxz_sb.rearrange("p (k m) -> p k m", k=8)
            wov = wo_sb.rearrange("p (k m) -> p k m", k=2)

            # zero the conv pads
            for dt_ in range(2):
                xp3 = xpad[dt_].rearrange("p (s l) -> p s l", s=B)
                nc.vector.memset(xp3[:, :, 0:KC - 1], 0.0)

            # ---- phase 1: in_proj (k outer, 4 live psum accumulators) ----
            for tb in range(4):
                psx = [ps.tile([128, 512], F32, name=f"psx{i}", tag="ps")
                       for i in range(4)]
                for k in range(8):
                    hkt = stream.tile([128, 512], F16, name="hkt", tag="hst")
                    nc.sync.dma_start(
                        hkt[:], hsT[128 * k:128 * (k + 1), 512 * tb:512 * (tb + 1)])
                    for half in range(4):  # x0 x1 z0 z1
                        nc.tensor.matmul(
                            psx[half][:],
                            wxzv[:, k, 128 * half:128 * (half + 1)],
                            hkt[:],
                            start=(k == 0), stop=(k == 7))
                for half in range(4):
                    dt_ = half % 2
                    sl5 = slice(512 * tb, 512 * (tb + 1))
                    if half < 2:   # x -> padded conv layout
                        b_ = tb // 2
                        off = b_ * PADL + (KC - 1) + 512 * (tb % 2)
                        nc.scalar.copy(xpad[dt_][:, off:off + 512], psx[half][:])
                    else:          # z -> silu(z) = (tanh(z/2)+1) * (z/2)
                        sg = stream.tile([128, 512], F32, name="sg", tag="sg")
                        nc.scalar.activation(sg[:], psx[half][:], AF.Tanh, scale=0.5)
                        nc.scalar.activation(zs[dt_][:, sl5], psx[half][:],
                                             AF.Identity, scale=0.5)
                        nc.vector.scalar_tensor_tensor(
                            zs[dt_][:, sl5], sg[:], 1.0, zs[dt_][:, sl5],
                            op0=OP.add, op1=OP.mult)

            # ---- phase 2: depthwise causal conv + silu -> u ----
            for dt_ in range(2):
                eng = nc.vector
                xp3 = xpad[dt_].rearrange("p (s l) -> p s l", s=B)
                c0_3 = cy0[dt_].rearrange("p (s l) -> p s l", s=B)
                c1_3 = cy1[dt_].rearrange("p (s l) -> p s l", s=B)
                eng.tensor_scalar_mul(c0_3[:], xp3[:, :, 0:L], cwv[:, dt_, 0:1])
                abuf = [c0_3, c1_3]
                for k in range(1, KC):
                    eng.scalar_tensor_tensor(
                        abuf[k % 2][:], xp3[:, :, k:k + L], cwv[:, dt_, k:k + 1],
                        abuf[(k + 1) % 2][:], op0=OP.mult, op1=OP.add)
                accf = abuf[(KC - 1) % 2].rearrange("p s l -> p (s l)")  # cy1
                sgtf = abuf[KC % 2].rearrange("p s l -> p (s l)")        # cy0
                # u = (acc+cb) * sigmoid(acc+cb) = (tanh(acc/2+cb/2)+1)*(acc/2+cb/2)
                nc.scalar.activation(sgtf[:], accf[:], AF.Tanh, scale=0.5,
                                     bias=cbh_sb[:, dt_:dt_ + 1])
                nc.scalar.activation(u[dt_][:], accf[:], AF.Identity, scale=0.5,
                                     bias=cbh_sb[:, dt_:dt_ + 1])
                nc.vector.scalar_tensor_tensor(
                    u[dt_][:], sgtf[:], 1.0, u[dt_][:], op0=OP.add, op1=OP.mult)

            # ---- phase 3: x_proj partial (fp32) -> AllReduce#1 ----
            for tb in range(4):
                ps96 = ps.tile([96, 512], F32, name="ps96", tag="ps")
                for k in range(2):
                    nc.tensor.matmul(
                        ps96[:], xpw_sb[:, 96 * k:96 * (k + 1)],
                        u[k][:, 512 * tb:512 * (tb + 1)],
                        start=(k == 0), stop=(k == 1))
                st = stg.tile([96, 512], F32, name="st_xp", tag="xp")
                nc.scalar.copy(st[:], ps96[:])
                nc.sync.dma_start(ar1_in[:, 512 * tb:512 * (tb + 1)], st[:])
            nc.gpsimd.collective_compute(
                "AllReduce", OP.add,
                replica_groups=[list(range(NC_))],
                ins=[ar1_in[:]], outs=[ar1_out[:]])
            nc.sync.dma_start(xdbl[:], ar1_out[0:RNK, :])
            # B/C rows -> fp16 scratch in DRAM (broadcast source)
            for th in range(2):
                bcs = stg.tile([32, L], F32, name="bcs", tag="bcs", bufs=1)
                nc.sync.dma_start(bcs[:], ar1_out[RNK:RNK + 32, L * th:L * (th + 1)])
                nc.vector.tensor_copy(bc16_sb[:, L * th:L * (th + 1)], bcs[:])
            nc.sync.dma_start(bc16d[:], bc16_sb[:])

            # ---- phase 4: dt_proj (fp32); softplus = Ln(Exp(dt+2b)+1) ----
            for tb in range(4):
                for dt_ in range(2):
                    psd = ps.tile([128, 512], F32, name="psd", tag="ps")
                    nc.tensor.matmul(
                        psd[:], dtw_sb[:, 128 * dt_:128 * (dt_ + 1)],
                        xdbl[:, 512 * tb:512 * (tb + 1)],
                        start=True, stop=True)
                    nc.scalar.activation(
                        delta[dt_][:, 512 * tb:512 * (tb + 1)], psd[:],
                        AF.Exp, bias=db2_sb[:, dt_:dt_ + 1])
            for dt_ in range(2):   # delta = ln(e + 1), in place, one big op
                nc.scalar.activation(delta[dt_][:], delta[dt_][:], AF.Ln, bias=1.0)
                nc.vector.tensor_mul(du[dt_][:], delta[dt_][:], u[dt_][:])

            # ---- phase 5+6+7 per batch: scan -> gate -> out_proj -> RS ----
            for b_ in range(2):
                tsl = slice(L * b_, L * (b_ + 1))
                py = [psy.tile([128, 512], F32, name=f"py{i}", tag="psy")
                      for i in range(4)]
                for n in range(NST):
                    bbc = scp.tile([128, L], F16, name="bbc", tag="bbc")
                    nc.sync.dma_start(
                        bbc[:], bc16d[n:n + 1, tsl].broadcast_to((128, L)))
                    cbc = scp.tile([128, L], F16, name="cbc", tag="cbc")
                    nc.sync.dma_start(
                        cbc[:], bc16d[NST + n:NST + n + 1, tsl].broadcast_to((128, L)))
                    for dt_ in range(2):
                        dA = scp.tile([128, L], F32, name="dA", tag=f"dA{dt_}")
                        nc.scalar.activation(
                            dA[:], delta[dt_][:, tsl], AF.Exp,
                            scale=av_sb[:, NST * dt_ + n:NST * dt_ + n + 1])
                        dbu = scp.tile([128, L], BF16, name="dbu", tag=f"dbu{dt_}")
                        nc.vector.tensor_mul(dbu[:], du[dt_][:, tsl], bbc[:])
                        h = scp.tile([128, L], F32, name="h", tag=f"h{dt_}")
                        nc.vector.tensor_tensor_scan(
                            h[:], dA[:], dbu[:], 0.0, op0=OP.mult, op1=OP.add)
                        ch = scp.tile([128, L], BF16, name="ch", tag=f"ch{dt_}")
                        cheng = nc.vector if dt_ == 0 else nc.gpsimd
                        cheng.tensor_mul(ch[:], h[:], cbc[:])
                        nc.tensor.matmul(py[2 * dt_][:], idb[:], ch[:, 0:512],
                                         start=(n == 0), stop=(n == NST - 1))
                        nc.tensor.matmul(py[2 * dt_ + 1][:], idb[:], ch[:, 512:L],
                                         start=(n == 0), stop=(n == NST - 1))
                for dt_ in range(2):
                    nc.scalar.copy(y[dt_][:, L * b_:L * b_ + 512], py[2 * dt_][:])
                    nc.scalar.copy(y[dt_][:, L * b_ + 512:L * (b_ + 1)],
                                   py[2 * dt_ + 1][:])
                    # skip + gate for this batch (gate output fp16 view of cy1)
                    nc.vector.scalar_tensor_tensor(
                        y[dt_][:, tsl], u[dt_][:, tsl], dv_sb[:, dt_:dt_ + 1],
                        y[dt_][:, tsl], op0=OP.mult, op1=OP.add)
                    nc.vector.tensor_mul(yg16[dt_][:, tsl], y[dt_][:, tsl],
                                         zs[dt_][:, tsl])
                # out_proj for this batch -> fp16 staging -> DRAM
                for tt in range(8):
                    t0 = L * b_ + 128 * tt
                    for mb in range(2):
                        pso = ps.tile([128, 512], F32, name="pso", tag="ps")
                        for k in range(2):
                            nc.tensor.matmul(
                                pso[:],
                                yg16[k][:, t0:t0 + 128],
                                wov[:, k, 512 * mb:512 * (mb + 1)],
                                start=(k == 0), stop=(k == 1))
                        st = stg.tile([128, 512], F16, name="st_op", tag="op")
                        nc.scalar.copy(st[:], pso[:])
                        nc.sync.dma_start(
                            ar2_in[b_][128 * tt:128 * (tt + 1),
                                       512 * mb:512 * (mb + 1)], st[:])
                nc.gpsimd.collective_compute(
                    "ReduceScatter", OP.add,
                    replica_groups=[list(range(NC_))],
                    ins=[ar2_in[b_][:]], outs=[ar2_out[b_][:]])
                nc.sync.dma_start(outp[b_], ar2_out[b_][:])
    nc.finalize()
    return nc


def make_in_maps(inputs):
    hs = np.asarray(inputs["hidden_states"], np.float32)
    ipw = np.asarray(inputs["in_proj_w"], np.float32)
    cw = np.asarray(inputs["conv_w"], np.float32)
    cb = np.asarray(inputs["conv_b"], np.float32)
    xpw = np.asarray(inputs["x_proj_w"], np.float32)
    dtw = np.asarray(inputs["dt_proj_w"], np.float32)
    dtb = np.asarray(inputs["dt_proj_b"], np.float32)
    alog = np.asarray(inputs["A_log"], np.float32)
    dvec = np.asarray(inputs["D"], np.float32)
    wo = np.asarray(inputs["out_proj_w"], np.float32)

    hsT = np.ascontiguousarray(hs.transpose(2, 0, 1).reshape(D, T)).astype(np.float16)
    ident = np.eye(128, dtype=np.float32)

    in_maps = []
    for c in range(NC_):
        sl = slice(DL * c, DL * (c + 1))
        wxzT = np.concatenate([ipw[sl].T, ipw[DI + DL * c: DI + DL * (c + 1)].T],
                              axis=1)                      # [1024, 512]
        m = {
            "hsT": hsT,
            "wxzT": np.ascontiguousarray(wxzT.reshape(8, 128, 512)).astype(np.float16),
            "xpwT": np.ascontiguousarray(xpw[:, sl].T.reshape(2, 128, 96)),
            "dtwT": np.ascontiguousarray(dtw[sl].T),        # [64, 256]
            "woT": np.ascontiguousarray(wo[:, sl].T.reshape(2, 128, D)).astype(np.float16),
            "convw": np.ascontiguousarray(cw[sl, 0, :].reshape(2, 128, KC)),
            "convbh": np.ascontiguousarray((0.5 * cb[sl]).reshape(2, 128, 1)),
            "dtb2p": np.ascontiguousarray((2.0 * dtb[sl]).reshape(2, 128, 1)),
            "Aneg": np.ascontiguousarray((-np.exp(alog[sl])).reshape(2, 128, NST)),
            "Dvec": np.ascontiguousarray(dvec[sl].reshape(2, 128, 1)),
            "ident": ident,
        }
        in_maps.append(m)
    return in_maps


def assemble_output(results):
    out = np.zeros((B, L, D), np.float32)
    for c in range(NC_):
        s = np.asarray(results[c]["out"], np.float32)  # [B, 128, D]
        for b_ in range(B):
            out[b_, 128 * c:128 * (c + 1), :] = s[b_]
    return out


def kernel(**inputs):
    from concourse.bass_utils import run_bass_kernel_spmd
    nc = build_nc()
    in_maps = make_in_maps(inputs)
    trace = bool(int(os.environ.get("MAMBA_TRACE", "0")))
    res = run_bass_kernel_spmd(nc, in_maps, list(range(NC_)), trace=trace)
    if trace and res.exec_time_ns is not None:
        print(f"HW exec time: {res.exec_time_ns} ns")
    return assemble_output(res.results)


# revision 23
# speedup vs baseline: 2.4528x; 1.0610x over previous
"""Mamba block Trainium2 kernel, 8-way tensor-parallel over d_inner.

Shapes (hardcoded from the problem spec):
  hidden_states [2, 1024, 1024], d_model=1024, d_inner=2048, d_state=16,
  dt_rank=64, d_conv=4.  Each core owns DL=256 d_inner channels.

Per-core dataflow:
  in_proj (fp16 matmuls, fp32 accumulate) -> x (padded conv layout), z
  silu via tanh: silu(v) = (tanh(v/2)+1) * (v/2)    [exp_and_others set]
  causal depthwise conv (DVE shifted MACs) + silu -> u
  x_proj partial (fp32) -> AllReduce#1 [96, 2048];  B/C rows -> fp16 DRAM
  dt_proj (fp32); softplus = Ln(Exp(dt+2b) + 1)     [ln/exp sets]
  per (b, n): B_bc/C_bc fp16 row-broadcast DMA (reused by both d-tiles)
     per d-tile: dA = exp(-exp(A_log)[:,n]*delta) (ACT per-partition scale)
                 dbu = du*B_bc (DVE 2x);  h = scan(dA, dbu) (DVE native scan)
                 ch = h*C_bc (DVE/POOL);  psum_y += I @ ch (PE bf16)
  per b: y = (u*D + y) * silu(z) -> fp16; out_proj (fp16) ->
         ReduceScatter fp16 -> per-core [128, 1024] slice of each batch
"""
import sys, os
sys.path.insert(0, "/opt/trn_rl_repo")
import numpy as np

import concourse.bass as bass
import concourse.bacc as bacc
import concourse.mybir as mybir
import concourse.tile as tile

F32 = mybir.dt.float32
F16 = mybir.dt.float16
BF16 = mybir.dt.bfloat16
AF = mybir.ActivationFunctionType
OP = mybir.AluOpType

B, L, D, DI, NST, RNK, KC = 2, 1024, 1024, 2048, 16, 64, 4
NC_ = 8
DL = DI // NC_          # 256 local channels
T = B * L               # 2048 tokens


def build_nc():
    nc = bacc.Bacc()
    dp = nc.declare_dram_parameter
    hsT = dp("hsT", [D, T], F16, isOutput=False)             # hidden^T fp16
    wxz = dp("wxzT", [8, 128, 512], F16, isOutput=False)     # in_proj^T k-tiles
    xpw = dp("xpwT", [2, 128, 96], F32, isOutput=False)      # x_proj^T k-tiles
    dtw = dp("dtwT", [RNK, DL], F16, isOutput=False)         # dt_proj^T
    wo = dp("woT", [2, 128, D], F16, isOutput=False)         # out_proj^T k-tiles
    cw = dp("convw", [2, 128, KC], F32, isOutput=False)
    cbh = dp("convbh", [2, 128, 1], F32, isOutput=False)     # conv_b / 2
    db2 = dp("dtb2p", [2, 128, 1], F32, isOutput=False)      # +2*dt_proj_b
    av = dp("Aneg", [2, 128, NST], F32, isOutput=False)      # -exp(A_log)
    dv = dp("Dvec", [2, 128, 1], F32, isOutput=False)
    idm = dp("ident", [128, 128], F32, isOutput=False)
    outp = dp("out", [B, 128, D], F16, isOutput=True)        # per-core RS slice

    ar1_in = [nc.dram_tensor(f"ar1_in{b}", [96, L], F16) for b in range(B)]
    ar1_out = [nc.dram_tensor(f"ar1_out{b}", [96, L], F16, addr_space="Shared")
               for b in range(B)]
    bc16d = nc.dram_tensor("bc16d", [32, T], F16)            # B/C rows interleaved
    ar2_in = [nc.dram_tensor(f"ar2_in{b}", [L, D], F16) for b in range(B)]
    ar2_out = [nc.dram_tensor(f"ar2_out{b}", [128, D], F16) for b in range(B)]

    PADL = L + KC - 1  # 1027 per batch segment

    with tile.TileContext(nc) as tc:
        with tc.tile_pool(name="wp", bufs=1) as wp, \
             tc.tile_pool(name="data", bufs=1) as dpool, \
             tc.tile_pool(name="stream", bufs=3) as stream, \
             tc.tile_pool(name="scan", bufs=2) as scp, \
             tc.tile_pool(name="stage", bufs=2) as stg, \
             tc.tile_pool(name="ps", bufs=4, space="PSUM") as ps, \
             tc.tile_pool(name="psy", bufs=4, space="PSUM") as psy:

            # ---- weights / constants ----
            wxz_sb = wp.tile([128, 8 * 512], F16)
            for k in range(8):
                nc.sync.dma_start(wxz_sb[:, 512 * k:512 * (k + 1)], wxz[k])
            xpw_sb = wp.tile([128, 2 * 96], F32)
            dtw_sb = wp.tile([RNK, DL], F16)
            nc.sync.dma_start(dtw_sb[:], dtw[:])
            wo_sb = wp.tile([128, 2 * D], F16)
            cw_sb = wp.tile([128, 2 * KC], F32)
            cbh_sb = wp.tile([128, 2], F32)
            db2_sb = wp.tile([128, 2], F32)
            av_sb = wp.tile([128, 2 * NST], F32)
            dv_sb = wp.tile([128, 2], F32)
            for k in range(2):
                nc.sync.dma_start(xpw_sb[:, 96 * k:96 * (k + 1)], xpw[k])
                nc.sync.dma_start(wo_sb[:, D * k:D * (k + 1)], wo[k])
                nc.sync.dma_start(cw_sb[:, KC * k:KC * (k + 1)], cw[k])
                nc.sync.dma_start(cbh_sb[:, k:k + 1], cbh[k])
                nc.sync.dma_start(db2_sb[:, k:k + 1], db2[k])
                nc.sync.dma_start(av_sb[:, NST * k:NST * (k + 1)], av[k])
                nc.sync.dma_start(dv_sb[:, k:k + 1], dv[k])
            id_sb = wp.tile([128, 128], F32)
            nc.sync.dma_start(id_sb[:], idm[:])
            idb = wp.tile([128, 128], BF16)
            nc.vector.tensor_copy(idb[:], id_sb[:])

            # ---- persistent activations ----
            xpad = [dpool.tile([128, B * PADL], F32, name=f"xpad{i}") for i in range(2)]
            zs = [dpool.tile([128, T], F32, name=f"zs{i}") for i in range(2)]
            u = [dpool.tile([128, T], F32, name=f"u{i}") for i in range(2)]
            delta = [dpool.tile([128, T], F32, name=f"delta{i}") for i in range(2)]
            du = [dpool.tile([128, T], F16, name=f"du{i}") for i in range(2)]
            # y reuses cy0; fp16 gated output aliases cy1's buffer (bitcast)
            cy0 = [dpool.tile([128, T], F32, name=f"cy0_{i}") for i in range(2)]
            cy1 = [dpool.tile([128, T], F32, name=f"cy1_{i}") for i in range(2)]
            y = cy0
            yg16 = [cy1[i].bitcast(F16)[:, 0:T] for i in range(2)]
            xdbl = [dpool.tile([RNK, L], F16, name=f"xdbl{i}") for i in range(2)]

            cwv = cw_sb.rearrange("p (k m) -> p k m", k=2)
            wxzv = wxz_sb.rearrange("p (k m) -> p k m", k=8)
            wov = wo_sb.rearrange("p (k m) -> p k m", k=2)

            # zero the conv pads
            for dt_ in range(2):
                xp3 = xpad[dt_].rearrange("p (s l) -> p s l", s=B)
                nc.vector.memset(xp3[:, :, 0:KC - 1], 0.0)

            # ---- phase 1: in_proj (k outer, 4 live psum accumulators) ----
            for tb in range(4):
                psx = [ps.tile([128, 512], F32, name=f"psx{i}", tag="ps")
                       for i in range(4)]
                for k in range(8):
                    hkt = stream.tile([128, 512], F16, name="hkt", tag="hst")
                    nc.sync.dma_start(
                        hkt[:], hsT[128 * k:128 * (k + 1), 512 * tb:512 * (tb + 1)])
                    for half in range(4):  # x0 x1 z0 z1
                        nc.tensor.matmul(
                            psx[half][:],
                            wxzv[:, k, 128 * half:128 * (half + 1)],
                            hkt[:],
                            start=(k == 0), stop=(k == 7))
                for half in range(4):
                    dt_ = half % 2
                    sl5 = slice(512 * tb, 512 * (tb + 1))
                    if half < 2:   # x -> padded conv layout
                        b_ = tb // 2
                        off = b_ * PADL + (KC - 1) + 512 * (tb % 2)
                        nc.scalar.copy(xpad[dt_][:, off:off + 512], psx[half][:])
                    else:          # z -> silu(z) = (tanh(z/2)+1) * (z/2)
                        sg = stream.tile([128, 512], F32, name="sg", tag="sg")
                        nc.scalar.activation(sg[:], psx[half][:], AF.Tanh, scale=0.5)
                        nc.scalar.activation(zs[dt_][:, sl5], psx[half][:],
                                             AF.Identity, scale=0.5)
                        nc.vector.scalar_tensor_tensor(
                            zs[dt_][:, sl5], sg[:], 1.0, zs[dt_][:, sl5],
                            op0=OP.add, op1=OP.mult)

            # ---- phase 2: depthwise causal conv + silu -> u ----
            for dt_ in range(2):
                eng = nc.vector
                xp3 = xpad[dt_].rearrange("p (s l) -> p s l", s=B)
                c0_3 = cy0[dt_].rearrange("p (s l) -> p s l", s=B)
                c1_3 = cy1[dt_].rearrange("p (s l) -> p s l", s=B)
                eng.tensor_scalar_mul(c0_3[:], xp3[:, :, 0:L], cwv[:, dt_, 0:1])
                abuf = [c0_3, c1_3]
                for k in range(1, KC):
                    eng.scalar_tensor_tensor(
                        abuf[k % 2][:], xp3[:, :, k:k + L], cwv[:, dt_, k:k + 1],
                        abuf[(k + 1) % 2][:], op0=OP.mult, op1=OP.add)
                accf = abuf[(KC - 1) % 2].rearrange("p s l -> p (s l)")  # cy1
                sgtf = abuf[KC % 2].rearrange("p s l -> p (s l)")        # cy0
                # u = (acc+cb) * sigmoid(acc+cb) = (tanh(acc/2+cb/2)+1)*(acc/2+cb/2)
                nc.scalar.activation(sgtf[:], accf[:], AF.Tanh, scale=0.5,
                                     bias=cbh_sb[:, dt_:dt_ + 1])
                nc.scalar.activation(u[dt_][:], accf[:], AF.Identity, scale=0.5,
                                     bias=cbh_sb[:, dt_:dt_ + 1])
                nc.vector.scalar_tensor_tensor(
                    u[dt_][:], sgtf[:], 1.0, u[dt_][:], op0=OP.add, op1=OP.mult)

            # ---- phase 3: x_proj partial -> per-batch fp16 AllReduce#1 ----
            bc16v = bc16d.rearrange("(r two) t -> two r t", two=2)
            for b_ in range(2):
                for th in range(2):
                    tb = 2 * b_ + th
                    ps96 = ps.tile([96, 512], F32, name="ps96", tag="ps")
                    for k in range(2):
                        nc.tensor.matmul(
                            ps96[:], xpw_sb[:, 96 * k:96 * (k + 1)],
                            u[k][:, 512 * tb:512 * (tb + 1)],
                            start=(k == 0), stop=(k == 1))
                    st = stg.tile([96, 512], F16, name="st_xp", tag="xp")
                    nc.scalar.copy(st[:], ps96[:])
                    nc.sync.dma_start(ar1_in[b_][:, 512 * th:512 * (th + 1)], st[:])
                if os.environ.get("MAMBA_NO_AR"):
                    nc.sync.dma_start(ar1_out[b_][:], ar1_in[b_][:])
                else:
                    nc.gpsimd.collective_compute(
                        "AllReduce", OP.add,
                        replica_groups=[list(range(NC_))],
                        ins=[ar1_in[b_][:]], outs=[ar1_out[b_][:]])
                nc.sync.dma_start(xdbl[b_][:], ar1_out[b_][0:RNK, :])
                tsl = slice(L * b_, L * (b_ + 1))
                nc.sync.dma_start(bc16v[0][:, tsl], ar1_out[b_][RNK:RNK + NST, :])
                nc.sync.dma_start(bc16v[1][:, tsl],
                                  ar1_out[b_][RNK + NST:RNK + 2 * NST, :])

            # ---- per batch: dt_proj/softplus -> scan -> gate -> out_proj -> RS
            for b_ in range(2):
                tsl = slice(L * b_, L * (b_ + 1))
                for th in range(2):
                    for dt_ in range(2):
                        psd = ps.tile([128, 512], F32, name="psd", tag="ps")
                        nc.tensor.matmul(
                            psd[:], dtw_sb[:, 128 * dt_:128 * (dt_ + 1)],
                            xdbl[b_][:, 512 * th:512 * (th + 1)],
                            start=True, stop=True)
                        nc.scalar.activation(
                            delta[dt_][:, L * b_ + 512 * th:L * b_ + 512 * (th + 1)],
                            psd[:], AF.Exp, bias=db2_sb[:, dt_:dt_ + 1])
                for dt_ in range(2):   # delta = ln(e + 1) in place; du = delta*u
                    nc.scalar.activation(delta[dt_][:, tsl], delta[dt_][:, tsl],
                                         AF.Ln, bias=1.0)
                    nc.vector.tensor_mul(du[dt_][:, tsl], delta[dt_][:, tsl],
                                         u[dt_][:, tsl])
                py = [psy.tile([128, 512], F32, name=f"py{i}", tag="psy")
                      for i in range(4)]
                for n in range(NST):
                    bct = scp.tile([128, 2 * L], F16, name="bct", tag="bct")
                    bct3 = bct.rearrange("p (two l) -> p two l", two=2)
                    nc.sync.dma_start(
                        bct3[:], bc16d[2 * n:2 * n + 2, tsl].unsqueeze(0).broadcast_to((128, 2, L)))
                    bbc = bct[:, 0:L]
                    cbc = bct[:, L:2 * L]
                    for dt_ in range(2):
                        dA = scp.tile([128, L], F32, name="dA", tag=f"dA{dt_}")
                        nc.scalar.activation(
                            dA[:], delta[dt_][:, tsl], AF.Exp,
                            scale=av_sb[:, NST * dt_ + n:NST * dt_ + n + 1])
                        dbu = scp.tile([128, L], BF16, name="dbu", tag=f"dbu{dt_}")
                        dbeng = nc.vector if dt_ == 0 else nc.gpsimd
                        dbeng.tensor_mul(dbu[:], du[dt_][:, tsl], bbc[:])
                        h = scp.tile([128, L], F32, name="h", tag=f"h{dt_}")
                        nc.vector.tensor_tensor_scan(
                            h[:], dA[:], dbu[:], 0.0, op0=OP.mult, op1=OP.add)
                        ch = scp.tile([128, L], BF16, name="ch", tag=f"ch{dt_}")
                        nc.gpsimd.tensor_mul(ch[:], h[:], cbc[:])
                        nc.tensor.matmul(py[2 * dt_][:], idb[:], ch[:, 0:512],
                                         start=(n == 0), stop=(n == NST - 1))
                        nc.tensor.matmul(py[2 * dt_ + 1][:], idb[:], ch[:, 512:L],
                                         start=(n == 0), stop=(n == NST - 1))
                for dt_ in range(2):
                    nc.scalar.copy(y[dt_][:, L * b_:L * b_ + 512], py[2 * dt_][:])
                    nc.scalar.copy(y[dt_][:, L * b_ + 512:L * (b_ + 1)],
                                   py[2 * dt_ + 1][:])
                    # skip + gate for this batch (gate output fp16 view of cy1)
                    nc.vector.scalar_tensor_tensor(
                        y[dt_][:, tsl], u[dt_][:, tsl], dv_sb[:, dt_:dt_ + 1],
                        y[dt_][:, tsl], op0=OP.mult, op1=OP.add)
                    nc.gpsimd.tensor_mul(yg16[dt_][:, tsl], y[dt_][:, tsl],
                                          zs[dt_][:, tsl])
                # out_proj for this batch -> fp16 staging -> DRAM
                for tt in range(8):
                    t0 = L * b_ + 128 * tt
                    for mb in range(2):
                        pso = ps.tile([128, 512], F32, name="pso", tag="ps")
                        for k in range(2):
                            nc.tensor.matmul(
                                pso[:],
                                yg16[k][:, t0:t0 + 128],
                                wov[:, k, 512 * mb:512 * (mb + 1)],
                                start=(k == 0), stop=(k == 1))
                        st = stg.tile([128, 512], F16, name="st_op", tag="op")
                        nc.scalar.copy(st[:], pso[:])
                        nc.sync.dma_start(
                            ar2_in[b_][128 * tt:128 * (tt + 1),
                                       512 * mb:512 * (mb + 1)], st[:])
                if os.environ.get("MAMBA_NO_AR"):
                    nc.sync.dma_start(ar2_out[b_][:], ar2_in[b_][0:128, :])
                else:
                    nc.gpsimd.collective_compute(
                        "ReduceScatter", OP.add,
                        replica_groups=[list(range(NC_))],
                        ins=[ar2_in[b_][:]], outs=[ar2_out[b_][:]])
                nc.sync.dma_start(outp[b_], ar2_out[b_][:])
    nc.finalize()
    return nc


def make_in_maps(inputs):
    hs = np.asarray(inputs["hidden_states"], np.float32)
    ipw = np.asarray(inputs["in_proj_w"], np.float32)
    cw = np.asarray(inputs["conv_w"], np.float32)
    cb = np.asarray(inputs["conv_b"], np.float32)
    xpw = np.asarray(inputs["x_proj_w"], np.float32)
    dtw = np.asarray(inputs["dt_proj_w"], np.float32)
    dtb = np.asarray(inputs["dt_proj_b"], np.float32)
    alog = np.asarray(inputs["A_log"], np.float32)
    dvec = np.asarray(inputs["D"], np.float32)
    wo = np.asarray(inputs["out_proj_w"], np.float32)

    hsT = np.ascontiguousarray(hs.transpose(2, 0, 1).reshape(D, T)).astype(np.float16)
    ident = np.eye(128, dtype=np.float32)

    in_maps = []
    for c in range(NC_):
        sl = slice(DL * c, DL * (c + 1))
        wxzT = np.concatenate([ipw[sl].T, ipw[DI + DL * c: DI + DL * (c + 1)].T],
                              axis=1)                      # [1024, 512]
        m = {
            "hsT": hsT,
            "wxzT": np.ascontiguousarray(wxzT.reshape(8, 128, 512)).astype(np.float16),
            "xpwT": np.ascontiguousarray(xpw[:, sl].T.reshape(2, 128, 96)),
            "dtwT": np.ascontiguousarray(dtw[sl].T).astype(np.float16),
            "woT": np.ascontiguousarray(wo[:, sl].T.reshape(2, 128, D)).astype(np.float16),
            "convw": np.ascontiguousarray(cw[sl, 0, :].reshape(2, 128, KC)),
            "convbh": np.ascontiguousarray((0.5 * cb[sl]).reshape(2, 128, 1)),
            "dtb2p": np.ascontiguousarray((2.0 * dtb[sl]).reshape(2, 128, 1)),
            "Aneg": np.ascontiguousarray((-np.exp(alog[sl])).reshape(2, 128, NST)),
            "Dvec": np.ascontiguousarray(dvec[sl].reshape(2, 128, 1)),
            "ident": ident,
        }
        in_maps.append(m)
    return in_maps


def assemble_output(results):
    out = np.zeros((B, L, D), np.float32)
    for c in range(NC_):
        s = np.asarray(results[c]["out"], np.float32)  # [B, 128, D]
        for b_ in range(B):
            out[b_, 128 * c:128 * (c + 1), :] = s[b_]
    return out


def kernel(**inputs):
    from concourse.bass_utils import run_bass_kernel_spmd
    nc = build_nc()
    in_maps = make_in_maps(inputs)
    trace = bool(int(os.environ.get("MAMBA_TRACE", "0")))
    res = run_bass_kernel_spmd(nc, in_maps, list(range(NC_)), trace=trace)
    if trace and res.exec_time_ns is not None:
        print(f"HW exec time: {res.exec_time_ns} ns")
    return assemble_output(res.results)
